# revision 1
# baseline (speedup 1.0000x reference)
"""Multi-head attention (B=2, S=2048, D=1024, H=16) on 8 NeuronCores. v3.

Sharding: core c -> batch c//4, head-group c%4 (4 heads, 256 proj dims).

Design notes (evolved from the 400us v1 baseline; v2 measured 295us):
- ACT (scalar engine) exp is the hard floor: 128 exps of [128,1024]
  ~147us. The attention stream keeps ACT 100% dense: two alternating
  PSUM score regions with a TWO-unit scores lookahead (scores for unit
  i+2 are emitted right after exp(i)), so the next exp's input is
  always ready, even across head-block boundaries and out-proj lumps.
- Single-head units (qc, h, kt): PSUM = s0(2) + s1(2) + ot(2) +
  op0(1) + op1(1) = exactly 8 banks.
- HAM: the PE clock-gate re-throttles to 1.2GHz if PE duty drops (v2
  lost 72us to this). Each scores emission carries one redundant
  half-score matmul (idempotent overwrite) sized to keep PE duty ~100%
  while ACT-bound; PE warm-up dummies cover the DMA head.
- bf16 inputs/weights: halves DMA-in; FWL on all weight loads. fp8 was
  rejected: its ~2-3% quantization lands ~1:1 on the output (random-
  walk sums), blowing the 2e-2 budget.
- V projected directly into (k-positions, head-dim) layout using the
  x-tile as stationary operand (no PE transposes). V bias folded into
  bo on the host (exact: attention rows sum to 1).
- Softmax normalize per block: reciprocal_approx_fast on the
  denominator row read straight from PSUM, one bf16 evac copy (frees
  the PV banks in ~2.4us), gpsimd partition-broadcast, bf16 mul.
- Q chunk-1 projection and the out-projection run as small lumps
  inside the attention stream's PE slack.
"""

import sys

sys.path.insert(0, "/opt/trn_rl_repo")

from contextlib import ExitStack

import numpy as np

import concourse.bacc as bacc
import concourse.mybir as mybir
import concourse.tile as tile
from concourse.bass_utils import run_bass_kernel_spmd

B = 2
S = 2048
D = 1024
H = 16
HD = 64
HPC = 4          # heads per core
DPC = HPC * HD   # 256 projection dims per core
NCORES = 8
SCALE = 8.0      # sqrt(HD)

F32 = mybir.dt.float32
BF16 = mybir.dt.bfloat16

DCH = D // 128   # 8 contraction chunks of 128
QT = S // 128    # 16 k-tiles of 128
QCN = 2          # q-chunks of 1024
QCW = 1024

EXPF = mybir.ActivationFunctionType.Exp


def build_nc():
    nc = bacc.Bacc("TRN2", target_bir_lowering=False, debug=False, num_devices=NCORES)

    xq = nc.dram_tensor("xq_t", [D, S], BF16, kind="ExternalInput")
    xk = nc.dram_tensor("xk_t", [D, S], BF16, kind="ExternalInput")
    xv = nc.dram_tensor("xv_t", [D, S], BF16, kind="ExternalInput")
    wq = nc.dram_tensor("wq_t", [D, DPC], BF16, kind="ExternalInput")
    wk = nc.dram_tensor("wk_t", [D, DPC], BF16, kind="ExternalInput")
    wv = nc.dram_tensor("wv_t", [D, DPC], BF16, kind="ExternalInput")
    wo = nc.dram_tensor("wo_t", [DPC, D], BF16, kind="ExternalInput")
    bq = nc.dram_tensor("bq", [DPC, 1], F32, kind="ExternalInput")
    bk = nc.dram_tensor("bk", [DPC, 1], F32, kind="ExternalInput")
    y = nc.dram_tensor("y", [S, D], BF16, kind="ExternalOutput")

    with tile.TileContext(nc) as tc, ExitStack() as ctx:
        const = ctx.enter_context(tc.tile_pool(name="const", bufs=1))
        xkp = ctx.enter_context(tc.tile_pool(name="xkp", bufs=1))
        xvp = ctx.enter_context(tc.tile_pool(name="xvp", bufs=1))
        xqp = ctx.enter_context(tc.tile_pool(name="xqp", bufs=1))
        qkv = ctx.enter_context(tc.tile_pool(name="qkv", bufs=1))
        ptp = ctx.enter_context(tc.tile_pool(name="ptp", bufs=1))
        nrm = ctx.enter_context(tc.tile_pool(name="nrm", bufs=2))
        yp = ctx.enter_context(tc.tile_pool(name="yp", bufs=3))

        # ---- t=0: ACT exp-table preload + PE warm-up fodder ----
        dmy = const.tile([1, 16], F32, tag="dmy")
        nc.vector.memset(dmy[:], 0.0)
        dmy2 = const.tile([1, 16], F32, tag="dmy2")
        nc.scalar.activation(dmy2[:], dmy[:], EXPF)

        wrm32 = const.tile([128, 128], F32, tag="wrm32")
        nc.vector.memset(wrm32[:], 0.0)
        wrm = const.tile([128, 512], BF16, tag="wrm")
        nc.vector.tensor_copy(wrm[:, 0:128], wrm32[:])

        onesv32 = const.tile([128, HPC], F32, tag="onesv32")
        nc.vector.memset(onesv32[:], 1.0)

        # ---- DMA issue order = consumption order: (wk,xk), (wv,xv),
        # (wq,wo,biases), xq half0, xq half1 ----
        wqc = const.tile([128, DCH, DPC], BF16, tag="wqc")
        wkc = const.tile([128, DCH, DPC], BF16, tag="wkc")
        wvc = const.tile([128, DCH, DPC], BF16, tag="wvc")
        woc = const.tile([128, 2, D], BF16, tag="woc")
        bqc = const.tile([128, 2, 1], F32, tag="bqc")
        bkc = const.tile([128, 2, 1], F32, tag="bkc")
        wq_sb = [wqc[:, d, :] for d in range(DCH)]
        wk_sb = [wkc[:, d, :] for d in range(DCH)]
        wv_sb = [wvc[:, d, :] for d in range(DCH)]
        wo_sb = [woc[:, g, :] for g in range(2)]
        bq_sb = [bqc[:, hp, :] for hp in range(2)]
        bk_sb = [bkc[:, hp, :] for hp in range(2)]
        xk2_sb = [xkp.tile([128, 2, S], BF16, tag=f"xk{d}", name=f"xk{d}") for d in range(DCH // 2)]
        xv2_sb = [xvp.tile([128, 2, S], BF16, tag=f"xv{d}", name=f"xv{d}") for d in range(DCH // 2)]
        xq4_sb = {(d4, hf): xqp.tile([128, 4, QCW], BF16, tag=f"xq{d4}_{hf}", name=f"xq{d4}_{hf}")
                  for hf in range(2) for d4 in range(DCH // 4)}
        xk_sb = [xk2_sb[d // 2][:, d % 2, :] for d in range(DCH)]
        xv_sb = [xv2_sb[d // 2][:, d % 2, :] for d in range(DCH)]
        xq_sb = {(d, hf): xq4_sb[(d // 4, hf)][:, d % 4, :]
                 for hf in range(2) for d in range(DCH)}

        # weights/biases first (they gate early compute) and COMBINED into
        # one transfer each: the ~0.6us per-DMA issue cost on the sync queue
        # otherwise serializes 22 small transfers into a ~14us trickle
        nc.sync.dma_start(bkc[:], bk[:, :].rearrange("(c p) o -> p c o", c=2))
        nc.sync.dma_start(bqc[:], bq[:, :].rearrange("(c p) o -> p c o", c=2))
        nc.sync.dma_start(wkc[:], wk[:, :].rearrange("(c p) m -> p c m", c=DCH))
        nc.sync.dma_start(wvc[:], wv[:, :].rearrange("(c p) m -> p c m", c=DCH))
        nc.sync.dma_start(wqc[:], wq[:, :].rearrange("(c p) m -> p c m", c=DCH))
        nc.sync.dma_start(woc[:], wo[:, :].rearrange("(c p) m -> p c m", c=2))
        # x streams in consumption order, two d-chunks per transfer so each
        # partition line is 8KB (DMA efficiency)
        for d2 in range(DCH // 2):
            nc.sync.dma_start(
                xk2_sb[d2][:],
                xk[d2 * 256:(d2 + 1) * 256, :].rearrange("(c p) s -> p c s", c=2))
        for d2 in range(DCH // 2):
            nc.sync.dma_start(
                xv2_sb[d2][:],
                xv[d2 * 256:(d2 + 1) * 256, :].rearrange("(c p) s -> p c s", c=2))
        for hf in range(2):
            for d4 in range(DCH // 4):
                nc.sync.dma_start(
                    xq4_sb[(d4, hf)][:],
                    xq[d4 * 512:(d4 + 1) * 512, hf * QCW:(hf + 1) * QCW]
                    .rearrange("(c p) s -> p c s", c=4))

        # ---- SBUF destinations ----
        kt_sb = [qkv.tile([128, S], BF16, tag=f"kt{hp}", name=f"ktt{hp}") for hp in range(2)]
        qt_sb = [qkv.tile([128, S], BF16, tag=f"qt{hp}", name=f"qtt{hp}") for hp in range(2)]
        # v tiles: [k-positions, 4 heads x (64 dims + ones col)]
        v_sb = [qkv.tile([128, HPC * (HD + 1)], BF16, tag=f"v{st}", name=f"v{st}") for st in range(QT)]
        for st in range(QT):
            v4 = v_sb[st][:].rearrange("p (h w) -> p h w", h=HPC)
            nc.vector.tensor_copy(
                v4[:, :, HD:HD + 1],
                onesv32[:].rearrange("p (a b) -> p a b", b=1),
            )
        otn_sb = [qkv.tile([128, S], BF16, tag=f"otn{j}", name=f"otn{j}") for j in range(2)]

        # ================= phase 1: K and V projections =================
        with tc.tile_pool(name="ps_p", bufs=1, space="PSUM") as ps_p:
            # PE warm-up: harmless matmuls on zero data while DMA streams in
            for i in range(84):
                wps = ps_p.tile([128, 512], F32, tag=f"pp{i % 8}", name=f"warm{i}")
                nc.tensor.matmul(wps[:], wrm[:, 0:128], wrm[:], start=True, stop=True)

            # K projection: weight-stationary, d' on partitions
            accs = {}
            for hp in range(2):
                for pc in range(4):
                    accs[(hp, pc)] = ps_p.tile([128, 512], F32, tag=f"pp{hp * 4 + pc}",
                                               name=f"ppk{hp}{pc}")
            for d in range(DCH):
                for hp in range(2):
                    for pc in range(4):
                        nc.tensor.matmul(
                            accs[(hp, pc)][:],
                            wk_sb[d][:, hp * 128:(hp + 1) * 128],
                            xk_sb[d][:, pc * 512:(pc + 1) * 512],
                            start=(d == 0), stop=(d == DCH - 1),
                        )
            for hp in range(2):
                for pc in range(4):
                    nc.vector.tensor_scalar_add(
                        kt_sb[hp][:, pc * 512:(pc + 1) * 512],
                        accs[(hp, pc)][:], bk_sb[hp])

            # V projection: x-tile stationary -> output [s-tile, 256 dims];
            # 2 rounds of 8 s-tiles (one PSUM bank each). No bias (folded
            # into bo on host).
            for rnd in range(2):
                vaccs = {}
                for sti in range(8):
                    st = rnd * 8 + sti
                    vaccs[st] = ps_p.tile([128, DPC], F32, tag=f"pp{sti}",
                                          name=f"ppv{st}")
                for d in range(DCH):
                    for sti in range(8):
                        st = rnd * 8 + sti
                        nc.tensor.matmul(
                            vaccs[st][:],
                            xv_sb[d][:, st * 128:(st + 1) * 128],
                            wv_sb[d],
                            start=(d == 0), stop=(d == DCH - 1),
                        )
                for sti in range(8):
                    st = rnd * 8 + sti
                    v4 = v_sb[st][:].rearrange("p (h w) -> p h w", h=HPC)
                    nc.vector.tensor_copy(
                        v4[:, :, 0:HD],
                        vaccs[st][:].rearrange("p (h w) -> p h w", h=HPC),
                    )

        # ============ phase 2: Q projection chunk 0 + attention ============
        with tc.tile_pool(name="ps_m", bufs=1, space="PSUM") as ps_m:
            s_t = [ps_m.tile([128, QCW], F32, tag=f"s{par}", name=f"sreg{par}")
                   for par in range(2)]
            op_t = [ps_m.tile([128, 512], F32, tag=f"op{dc}", name=f"opreg{dc}")
                    for dc in range(2)]

            # PE warm bridge across the xq0 DMA wait (keeps HAM at 2.4GHz
            # between the V projection and the Q projection)
            for i in range(12):
                wbp = ps_m.tile([128, 512], F32, tag=f"op{i % 2}", name=f"warmq{i}")
                nc.tensor.matmul(wbp[:], wrm[:, 0:128], wrm[:], start=True, stop=True)

            # Q projection pass 1 (columns 0:1024) on the score-region banks
            for hp in range(2):
                for pc in range(2):
                    acc = s_t[hp][:, pc * 512:(pc + 1) * 512]
                    for d in range(DCH):
                        nc.tensor.matmul(
                            acc,
                            wq_sb[d][:, hp * 128:(hp + 1) * 128],
                            xq_sb[(d, 0)][:, pc * 512:(pc + 1) * 512],
                            start=(d == 0), stop=(d == DCH - 1),
                        )
            for hp in range(2):
                for pc in range(2):
                    nc.vector.tensor_scalar_add(
                        qt_sb[hp][:, pc * 512:(pc + 1) * 512],
                        s_t[hp][:, pc * 512:(pc + 1) * 512], bq_sb[hp])

            # attention unit stream: 128 units of (qc, h, kt)
            units = [(qc, h, kt) for qc in range(QCN) for h in range(HPC)
                     for kt in range(QT)]
            NU = len(units)
            # lumps of extra PE work inside the stream: Q-chunk-1
            # projection early, out-proj of q-chunk 0 spread over chunk 1
            # qp2 at block-START units: PV(kt=0) stalls ~2.4us on the prior
            # block's evac there, so the lump fills otherwise-dead PE time
            qp2_at = {16: 2, 32: 3, 48: 4, 80: 5}
            out_at = {64 + 8 * t + 7: t for t in range(8)}   # unit -> ytile

            def emit_scores(i):
                qc, h, kt = units[i]
                j, h2 = h // 2, h % 2
                sv = ps_m.tile([128, QCW], F32, tag=f"s{i % 2}", name=f"s_{i}")
                for half in range(2):
                    nc.tensor.matmul(
                        sv[:, half * 512:(half + 1) * 512],
                        kt_sb[j][h2 * 64:h2 * 64 + 64, kt * 128:(kt + 1) * 128],
                        qt_sb[j][h2 * 64:h2 * 64 + 64,
                                 qc * QCW + half * 512:qc * QCW + (half + 1) * 512],
                        start=True, stop=True,
                    )
                if i not in qp2_at and i not in out_at:
                    # redundant half-score rewrite: pure PE-duty filler so the
                    # HAM clock gate stays at 2.4GHz while ACT is the limiter
                    nc.tensor.matmul(
                        sv[:, 0:128],
                        kt_sb[j][h2 * 64:h2 * 64 + 64, kt * 128:(kt + 1) * 128],
                        qt_sb[j][h2 * 64:h2 * 64 + 64, qc * QCW:qc * QCW + 128],
                        start=True, stop=True,
                    )
                return sv

            def emit_qp2(lump):
                # Q projection pass 2 (columns 1024:2048), one [128,512]
                # output sub-column per lump on an out-proj PSUM bank
                hp, pc = (lump - 2) % 2, 2 + (lump - 2) // 2
                acc = ps_m.tile([128, 512], F32, tag=f"op{hp}", name=f"qp2_{pc}{hp}")
                for d in range(DCH):
                    nc.tensor.matmul(
                        acc[:],
                        wq_sb[d][:, hp * 128:(hp + 1) * 128],
                        xq_sb[(d, 1)][:, (pc - 2) * 512:(pc - 1) * 512],
                        start=(d == 0), stop=(d == DCH - 1),
                    )
                nc.vector.tensor_scalar_add(
                    qt_sb[hp][:, pc * 512:(pc + 1) * 512],
                    acc[:], bq_sb[hp])

            def emit_ytile(yt, tail=False):
                ysb = yp.tile([128, D], BF16, tag="y", name=f"ysb{yt}")
                for dc in range(2):
                    if tail:
                        # score-region banks are dead after the last exp: use
                        # their right halves as accumulators 3/4 so the 2-bank
                        # op rotation never stalls matmuls on evac copies
                        k = (yt * 2 + dc) % 4
                        if k < 2:
                            op = ps_m.tile([128, 512], F32, tag=f"op{k}",
                                           name=f"op{yt}{dc}")
                        else:
                            op = ps_m.tile([128, QCW], F32, tag=f"s{k - 2}",
                                           name=f"ops{yt}{dc}")[:, 512:1024]
                    else:
                        op = ps_m.tile([128, 512], F32, tag=f"op{dc}", name=f"op{yt}{dc}")
                    for g in range(2):
                        nc.tensor.matmul(
                            op[0:128, 0:512],
                            otn_sb[g][:, yt * 128:(yt + 1) * 128],
                            wo_sb[g][:, dc * 512:(dc + 1) * 512],
                            start=(g == 0), stop=(g == 1),
                        )
                    if tail and dc == 1:
                        # ACT is idle in the tail; split evac copies across engines
                        nc.scalar.copy(ysb[:, dc * 512:(dc + 1) * 512], op[0:128, 0:512])
                    else:
                        nc.vector.tensor_copy(ysb[:, dc * 512:(dc + 1) * 512], op[0:128, 0:512])
                    if tail:
                        # stream each half out as soon as its copy lands
                        nc.sync.dma_start(
                            y[yt * 128:(yt + 1) * 128, dc * 512:(dc + 1) * 512],
                            ysb[:, dc * 512:(dc + 1) * 512])
                if not tail:
                    nc.sync.dma_start(y[yt * 128:(yt + 1) * 128, :], ysb[:])

            def emit_norm(qc, h, ot, half=None):
                # normalize: otn[d,q] = ot[d,q] / ot[64,q]. half=0/1 splits
                # the chain by columns (used on the final block so the tail
                # out-projection can start after half 0).
                j, h2 = h // 2, h % 2
                cs = slice(None) if half is None else slice(half * 512, (half + 1) * 512)
                w = QCW if half is None else 512
                co = 0 if half in (None, 0) else 512
                drow = nrm.tile([1, QCW], F32, tag="drow", name=f"drow{qc}{h}{half}")
                nc.vector.tensor_copy(drow[:, 0:w], ot[HD:HD + 1, cs])
                otr = nrm.tile([HD, QCW], BF16, tag="otr", name=f"otr{qc}{h}{half}")
                nc.vector.tensor_copy(otr[:, 0:w], ot[0:HD, cs])
                r32 = nrm.tile([1, QCW], F32, tag="r32", name=f"r32{qc}{h}{half}")
                nc.vector.reciprocal_approx_fast(r32[:, 0:w], drow[:, 0:w])
                rb = nrm.tile([1, QCW], BF16, tag="rb", name=f"rb{qc}{h}{half}")
                nc.vector.tensor_copy(rb[:, 0:w], r32[:, 0:w])
                sc = nrm.tile([HD, QCW], BF16, tag="sc", name=f"sc{qc}{h}{half}")
                nc.gpsimd.partition_broadcast(sc[:, 0:w], rb[:, 0:w])
                nc.vector.tensor_mul(
                    otn_sb[j][h2 * 64:h2 * 64 + 64,
                              qc * QCW + co:qc * QCW + co + w],
                    otr[:, 0:w], sc[:, 0:w])

            sv = {0: emit_scores(0), 1: emit_scores(1)}
            ot = None
            for i, (qc, h, kt) in enumerate(units):
                if kt == 0:
                    ot = ps_m.tile([HD + 1, QCW], F32, tag="ot", name=f"ot{qc}{h}")
                pt = ptp.tile([128, QCW], BF16, tag=f"pt{i % 4}")
                nc.scalar.activation(pt[:], sv.pop(i)[:], EXPF, scale=1.0 / SCALE)
                if i + 2 < NU:
                    sv[i + 2] = emit_scores(i + 2)
                if i in qp2_at:
                    emit_qp2(qp2_at[i])
                for half in range(2):
                    nc.tensor.matmul(
                        ot[:, half * 512:(half + 1) * 512],
                        v_sb[kt][:, h * 65:h * 65 + 65],
                        pt[:, half * 512:(half + 1) * 512],
                        start=(kt == 0), stop=(kt == QT - 1),
                    )
                if kt == QT - 1:
                    if i == NU - 1:
                        emit_norm(qc, h, ot, half=0)
                        emit_norm(qc, h, ot, half=1)
                    else:
                        emit_norm(qc, h, ot)
                if i in out_at:
                    emit_ytile(out_at[i])
            # keep the PE busy while the last block's normalize chain runs,
            # so the HAM clock-gate stays warm for the tail out-projection
            for i in range(18):
                svd = ps_m.tile([128, QCW], F32, tag=f"s{i % 2}", name=f"warmt{i}")
                nc.tensor.matmul(svd[:, 0:512], wrm[:, 0:128], wrm[:],
                                 start=True, stop=True)
            for yt in range(8, 16):
                emit_ytile(yt, tail=True)

    nc.compile()
    return nc


_NC_CACHE = None


def _get_nc():
    global _NC_CACHE
    if _NC_CACHE is None:
        _NC_CACHE = build_nc()
    return _NC_CACHE


def shard_inputs(query, key, value, Wq, bq, Wk, bk, Wv, bv, Wo, bo):
    """Build the 8 per-core input maps (host-side shard + transpose)."""
    import ml_dtypes
    f = np.float32
    bf = ml_dtypes.bfloat16
    in_maps = []
    for c in range(NCORES):
        b = c // 4
        g = c % 4
        hs = slice(g * DPC, (g + 1) * DPC)
        in_maps.append({
            "xq_t": np.ascontiguousarray(np.asarray(query[b], f).T.astype(bf)),
            "xk_t": np.ascontiguousarray(np.asarray(key[b], f).T.astype(bf)),
            "xv_t": np.ascontiguousarray(np.asarray(value[b], f).T.astype(bf)),
            "wq_t": np.ascontiguousarray(np.asarray(Wq[hs, :], f).T.astype(bf)),
            "wk_t": np.ascontiguousarray(np.asarray(Wk[hs, :], f).T.astype(bf)),
            "wv_t": np.ascontiguousarray(np.asarray(Wv[hs, :], f).T.astype(bf)),
            "wo_t": np.ascontiguousarray(np.asarray(Wo[:, hs], f).T.astype(bf)),
            "bq": np.asarray(bq[hs], f).reshape(DPC, 1).copy(),
            "bk": np.asarray(bk[hs], f).reshape(DPC, 1).copy(),
        })
    return in_maps


def kernel(query, key, value, Wq, bq, Wk, bk, Wv, bv, Wo, bo, **run_kwargs):
    nc = _get_nc()
    in_maps = shard_inputs(query, key, value, Wq, bq, Wk, bk, Wv, bv, Wo, bo)
    res = run_bass_kernel_spmd(nc, in_maps, core_ids=list(range(NCORES)),
                               **run_kwargs)
    out = np.zeros((B, S, D), np.float32)
    for c in range(NCORES):
        out[c // 4] += np.asarray(res.results[c]["y"], np.float32)
    # V bias folded here: attention rows sum to 1, so +bv passes through
    # attention unchanged and contributes bv @ Wo.T to every output row.
    bo_eff = np.asarray(bo, np.float32) + np.asarray(bv, np.float32) @ np.asarray(Wo, np.float32).T
    out += bo_eff
    if run_kwargs:
        kernel.last_result = res
    return out

